# revision 33
# baseline (speedup 1.0000x reference)
"""Ewald reciprocal-space kernel for Trainium2 (8 NeuronCores, SPMD).

Math (per batch b):
    s        = cell_inv @ x          (fractional coords)
    theta    = 2*pi * (kvec . s)     (B, N, NK) phases
    S_re/S_im= sum_n q_n {cos,sin}(theta)          (structure factor)
    recip_n  = sum_k expfac_k (S_re cos + S_im sin)
    phi      = recip * BOHR/(pi*V) - q * 2*bewald*BOHR/sqrt(pi)
    returns (0.5*q*phi, phi)

Sharding: 8 cores = 2 batches x 4 k-shards (1024 k-vectors each). Each core
computes its full-N, shard-K contribution to recip with no collectives; host
sums the 4 shard partials per batch and applies the final affine.

Device pipeline per core (N=4096 as 32 chunks of 128 partitions):
  u = x . (Cinv^T k)  [= theta/2pi]   via fp32r matmul (contraction dim 3)
  rn = (u + M) - M            magic-number round-to-nearest (DVE tensor_scalar)
  -r = rn - u                 in [-1/2, 1/2]   (DVE scalar_tensor_tensor)
  -r_c = wrap(-r - 1/4)       in [-1/2, 1/2]   (DVE add_range_wrap custom op)
  sin(theta) = Sin(-2pi * -r), cos(theta) = Sin(-2pi * -r_c)  (ACT, fp16 out)
  S_re/S_im: PE matmuls contracting n with q as weights (psum accumulate)
  cs chunks DMA-transposed (xbar) into csT[k-slice partitions, n free]
  w = expfac * S  (small), transposed to a [128,16] column tile via DRAM bounce
  recip: PE matmuls contracting k-slices: sum_j wcol_j^T @ csT_j
"""

import math
from contextlib import ExitStack

import numpy as np

BOHR = 1.8897261258369282

B, N, NK = 2, 4096, 4096
NCORES = 8
KSH = NK // 4          # k-vectors per core
NCH = N // 128         # 32 n-chunks
CW = 2 * KSH           # cs chunk width: [cos | sin]
NSL = CW // 128        # 16 k-slices per chunk

_PROG = {}


def _build_program():
    import concourse.bass as bass
    import concourse.bacc as bacc
    import concourse.tile as tile
    import concourse.mybir as mybir

    F32 = mybir.dt.float32
    F32R = mybir.dt.float32r
    F16 = mybir.dt.float16
    MAGIC = 12582912.0          # 1.5 * 2**23: fp32 round-to-nearest-integer
    # two fp32 ulps below 2*pi so |scale * r| <= pi holds for r = +-1/2 exactly
    NEG2PI = -6.28318452835083
    ADD = mybir.AluOpType.add
    SUB = mybir.AluOpType.subtract

    nc = bacc.Bacc(trn_type="TRN2", target_bir_lowering=False, debug=False)

    coordsT_d = nc.dram_tensor("coordsT", [3, N], F32, kind="ExternalInput").ap()
    qT_d = nc.dram_tensor("qT", [128, NCH], F32, kind="ExternalInput").ap()
    cinv_d = nc.dram_tensor("cinv", [3, 3], F32, kind="ExternalInput").ap()
    kvecT_d = nc.dram_tensor("kvecT", [3, KSH], F32, kind="ExternalInput").ap()
    expfac_d = nc.dram_tensor("expfac", [1, KSH], F32, kind="ExternalInput").ap()
    recip_d = nc.dram_tensor("recip", [1, N], F32, kind="ExternalOutput").ap()
    wb_d = nc.dram_tensor("w_bounce", [1, CW], F16)

    with tile.TileContext(nc) as tc, ExitStack() as ctx:
        const = ctx.enter_context(tc.tile_pool(name="const", bufs=1))
        pu = ctx.enter_context(tc.tile_pool(name="pu", bufs=2, space="PSUM"))
        pacc = ctx.enter_context(tc.tile_pool(name="pacc", bufs=1, space="PSUM"))
        wk_rn = ctx.enter_context(tc.tile_pool(name="wk_rn", bufs=2))
        wk_mr = ctx.enter_context(tc.tile_pool(name="wk_mr", bufs=3))
        wk_cs = ctx.enter_context(tc.tile_pool(name="wk_cs", bufs=4))
        wk_out = ctx.enter_context(tc.tile_pool(name="wk_out", bufs=2))

        # ---- load inputs ----
        kvt = wk_cs.tile([3, KSH], F32R, tag="cs", name="kvt")
        nc.sync.dma_start(out=kvt[:, :], in_=kvecT_d.bitcast(F32R))
        cinv_t = const.tile([3, 3], F32R)
        nc.sync.dma_start(out=cinv_t[:, :], in_=cinv_d.bitcast(F32R))
        cts = const.tile([3, N], F32R)
        nc.sync.dma_start(out=cts[:, 0:1024], in_=coordsT_d[:, 0:1024].bitcast(F32R))
        qt = const.tile([128, NCH], F32)
        nc.sync.dma_start(out=qt[:, :], in_=qT_d)
        for h in range(1024, N, 1024):
            nc.sync.dma_start(
                out=cts[:, h : h + 1024],
                in_=coordsT_d[:, h : h + 1024].bitcast(F32R),
            )
        ef_a = wk_out.tile([1, 512], F32, tag="rr", name="ef_a")
        nc.sync.dma_start(out=ef_a[:, :], in_=expfac_d[:, 0:512])
        ef_b = wk_out.tile([1, 512], F32, tag="rr", name="ef_b")
        nc.sync.dma_start(out=ef_b[:, :], in_=expfac_d[:, 512:1024])

        qt16 = const.tile([128, NCH], F16)
        nc.scalar.copy(qt16[:, :], qt[:, :])

        # ---- kmodT[j, k] = sum_i cinv[i, j] * kvecT[i, k]  (= (Cinv^T k)^T) ----
        km_ps = pu.tile([128, KSH], F32, tag="u")
        for h in range(0, KSH, 512):
            nc.tensor.matmul(
                km_ps[:3, h : h + 512], lhsT=cinv_t[:, :], rhs=kvt[:, h : h + 512],
                start=True, stop=True,
            )
        kmod = const.tile([3, KSH], F32R)
        nc.vector.tensor_copy(kmod[:, :], km_ps[:3, :])

        # chunks whose round-to-nearest runs on the scalar engine (balances
        # DVE vs ACT busy time; ~19/32 assisted)
        ASSIST = {int((i + 0.5) * NCH / 19) for i in range(19)}

        # persistent stores
        csT = const.tile([128, NSL, N], F16)   # [k-in-slice][slice j][n]
        sab = [
            pacc.tile([1, 512], F32, tag=f"sab{j}", name=f"sab{j}") for j in range(4)
        ]

        # ---- pass 1: phases, trig, structure factors, transposes ----
        # Software-pipelined one chunk ahead: the (matmul -> round-to-nearest)
        # production for chunk t+1 is emitted before chunk t's Sin
        # activations, so the DVE's scalar_tensor_tensor never waits on a
        # busy scalar engine.
        def produce(t):
            u_ps = pu.tile([128, KSH], F32, tag="u", name=f"u{t}")
            for h in range(0, KSH, 512):
                nc.tensor.matmul(
                    u_ps[:, h : h + 512],
                    lhsT=cts[:, 128 * t : 128 * (t + 1)],
                    rhs=kmod[:, h : h + 512],
                    start=True, stop=True,
                )
            rn = wk_rn.tile([128, KSH], F32, tag="rn", name=f"rn{t}")
            if t in ASSIST:
                # scalar engine computes v = u + M; DVE then gets rn - u via
                # (v - M) - u in one scalar_tensor_tensor
                nc.scalar.activation(
                    rn[:, :], u_ps[:, :],
                    mybir.ActivationFunctionType.Copy, bias=MAGIC, scale=1.0,
                )
                s0 = MAGIC
            else:
                nc.vector.tensor_scalar(
                    out=rn[:, :], in0=u_ps[:, :], scalar1=MAGIC, scalar2=MAGIC,
                    op0=ADD, op1=SUB,
                )
                s0 = 0.0
            return u_ps, rn, s0

        cur = produce(0)
        for t in range(NCH):
            u_ps, rn, s0 = cur
            mm = wk_mr.tile([128, CW], F32)    # [-r | -r_c] halves
            nc.vector.scalar_tensor_tensor(
                out=mm[:, 0:KSH], in0=rn[:, :], scalar=s0, in1=u_ps[:, :],
                op0=ADD if s0 == 0.0 else SUB, op1=SUB,
            )
            nc.vector.add_range_wrap(
                out=mm[:, KSH:CW], in_=mm[:, 0:KSH],
                shift=-0.25, bound=0.5, period=1.0,
            )
            if t + 1 < NCH:
                cur = produce(t + 1)
            # one Sin over both halves: cs = [sin(theta) | cos(theta)]
            cs = wk_cs.tile([128, CW], F16, tag="cs")
            nc.scalar.activation(
                cs[:, :], mm[:, :],
                mybir.ActivationFunctionType.Sin, bias=0.0, scale=NEG2PI,
            )
            for j in range(4):
                nc.tensor.matmul(
                    sab[j][:, :],
                    lhsT=qt16[:, t : t + 1],
                    rhs=cs[:, 512 * j : 512 * (j + 1)],
                    start=(t == 0), stop=(t == NCH - 1),
                )
            # csT[p, j, 128t + n] = cs[n, 128j + p]
            nc.sync.dma_start_transpose(
                out=csT[:, :, 128 * t : 128 * (t + 1)], in_=cs[:, :],
            )

        # ---- mid: w = expfac * S; transpose to column layout via DRAM ----
        w_row = const.tile([1, CW], F16, tag="w_row")
        for j in range(4):
            nc.vector.tensor_tensor(
                out=w_row[:, 512 * j : 512 * (j + 1)],
                in0=sab[j][:, :],
                in1=(ef_a if j % 2 == 0 else ef_b)[:, :],
                op=mybir.AluOpType.mult,
            )
        nc.sync.dma_start(out=wb_d.ap(), in_=w_row[:, :])
        wcolT = const.tile([128, NSL], F16, tag="wcolT")
        nc.sync.dma_start_transpose(
            out=wcolT[:, :], in_=wb_d.ap().rearrange("a (j p) -> (a j) p", p=128),
        )

        # ---- pass 2: recip = sum_j wcol_j^T @ csT_j  (contract k on PE) ----
        for nf in range(0, N, 512):
            pb = pu.tile([1, 512], F32, tag="u", name="pb")
            for j in range(NSL):
                nc.tensor.matmul(
                    pb[:, :],
                    lhsT=wcolT[:, j : j + 1],
                    rhs=csT[:, j, nf : nf + 512],
                    start=(j == 0), stop=(j == NSL - 1),
                )
            rr = wk_out.tile([1, 512], F32)
            nc.scalar.copy(rr[:, :], pb[:, :])
            nc.sync.dma_start(out=recip_d[:, nf : nf + 512], in_=rr[:, :])

    nc.compile()
    return nc


def _get_prog(reps: int = 1, stage: str = "full"):
    key = (reps, stage)
    if key not in _PROG:
        _PROG[key] = _build_program(reps, stage)
    return _PROG[key]


def _make_in_maps(coords, q, cell_inv, kvec, expfac):
    in_maps = []
    for c in range(NCORES):
        b, ks = divmod(c, NCORES // B)
        sl = slice(KSH * ks, KSH * (ks + 1))
        in_maps.append({
            "coordsT": np.ascontiguousarray(coords[b].T, dtype=np.float32),
            "qT": np.ascontiguousarray(q[b].reshape(NCH, 128).T, dtype=np.float32),
            "cinv": np.ascontiguousarray(cell_inv, dtype=np.float32),
            "kvecT": np.ascontiguousarray(kvec[sl].T, dtype=np.float32),
            "expfac": np.ascontiguousarray(expfac[sl][None, :], dtype=np.float32),
        })
    return in_maps


def _finalize(results, q, volume, bewald):
    recip = np.zeros((B, N), np.float32)
    for c in range(NCORES):
        b = c // (NCORES // B)
        recip[b] += results[c]["recip"][0]
    scale1 = np.float32(BOHR / (math.pi * float(volume[0])))
    scale2 = np.float32(2.0 * float(bewald[0]) * BOHR / math.sqrt(math.pi))
    phi = (recip * scale1 - q.astype(np.float32) * scale2).astype(np.float32)
    e = (np.float32(0.5) * q.astype(np.float32) * phi).astype(np.float32)
    return e, phi


def kernel(coords, q, cell_inv, kvec, expfac, volume, bewald):
    from concourse.bass_utils import run_bass_kernel_spmd

    nc = _get_prog()
    in_maps = _make_in_maps(coords, q, cell_inv, kvec, expfac)
    res = run_bass_kernel_spmd(nc, in_maps, list(range(NCORES))).results
    return _finalize(res, q, volume, bewald)
